# revision 13
# baseline (speedup 1.0000x reference)
"""Trainium2 Bass kernel for nn_Attention2 (single-head attention, row-0 output).

The reference computes full attention out = softmax(q k^T / sqrt(d)) v per
(b, inst) pair and returns only out[:, :, 0, :], so only query row 0 matters:

    c   = x0 @ (Wq^T Wk) / sqrt(d)      # M1 = Wq^T Wk / sqrt(d) on host
    s_l = sum_d c_d x[l, d]             # scores (fused mul+reduce)
    e   = exp(s)                        # ACT
    ue  = e @ x                         # PE, [1,512] PSUM per pair
    out = (ue @ Wv^T) / sum(e)          # PE + ACT scale, once in the tail

v2 streams x as FLOAT16 (host-cast), halving the HBM floor from ~80 us to
~40 us per core, and splits the per-pair score work (4 tiles of [128,512]
mul+reduce, the vector-engine bottleneck) across THREE engines via the fused
scalar_tensor_tensor op (out=(in0*1)*in1, accum_out=row sums):

  DVE  : 2 score tiles (~0.64 us/tile)
  Pool : 2 score tiles (~0.8 us/tile, Q7 at 1.2 GHz x 0.6 eff)
  ACT  : cb PSUM->SBUF fp16 copy, exp, ue row PSUM->SBUF copy
  PE   : basis-selector broadcast of c row j + 4 accumulating fp16 matmuls
  SP   : one 512 KB HWDGE DMA per pair (fp16 x tile, 4 KB runs)

Per-engine steady state is ~40-52 us; DMA floor ~42 us.  8 cores are pure
data-parallel over the 256 (b, inst) pairs (32 pairs each).  fp16 end-to-end
rel err vs the fp32 reference is ~3e-4 (gate is 2e-2).
"""

import contextlib

import numpy as np

import concourse.tile as tile
from concourse import bacc, bass_utils, mybir
from concourse.bass import ts

F32 = mybir.dt.float32
R32 = mybir.dt.float32r
F16 = mybir.dt.float16

N_CORES = 8
B, INST, L, D = 8, 32, 512, 512
P = 128
LT = L // P  # 4 l-tiles
DT = D // P  # 4 d-tiles
PAIRS = (B * INST) // N_CORES  # 32 pairs per core

MULT = mybir.AluOpType.mult


def _build_program(pairs=PAIRS, mode="full", hw_reps=None, bufs_x=8,
                   lead=4, assign="DDDP", ue_copy="act", dma_chunk=1,
                   bufs_cb=None, bufs_s=4, ue_batch=2, dbg=False):
    nc = bacc.Bacc(
        "TRN2",
        target_bir_lowering=False,
        debug=False,
        num_devices=N_CORES,
    )
    if bufs_cb is None:
        bufs_cb = lead + 2

    selp = LT * pairs  # 128: rows of the lt-collapse selection matrix

    x_t = nc.dram_tensor("x", [pairs, L, D], F16, kind="ExternalInput")
    ident_t = nc.dram_tensor("ident", [P, P], F32, kind="ExternalInput")
    x0t_t = nc.dram_tensor("x0t", [D, pairs], R32, kind="ExternalInput")
    m1_t = nc.dram_tensor("m1", [D, D], R32, kind="ExternalInput")
    wvt_t = nc.dram_tensor("wvt", [D, D], R32, kind="ExternalInput")
    sel_t = nc.dram_tensor("sel", [selp, pairs], F32, kind="ExternalInput")
    ones2_t = nc.dram_tensor("ones2", [P, 2], F16, kind="ExternalInput")
    basis_t = nc.dram_tensor("basis", [pairs, pairs * P], F16,
                             kind="ExternalInput")
    out_t = nc.dram_tensor("out", [pairs, D], F32, kind="ExternalOutput")
    if dbg:
        dbg_c_t = nc.dram_tensor("dbg_c", [pairs, D], F32,
                                 kind="ExternalOutput")
        dbg_e_t = nc.dram_tensor("dbg_e", [P, LT * pairs], F32,
                                 kind="ExternalOutput")
        dbg_ue_t = nc.dram_tensor("dbg_ue", [1, pairs * D], F32,
                                  kind="ExternalOutput")

    x_ap = x_t.ap()

    with tile.TileContext(nc) as tc:
        with (
            tc.tile_pool(name="consts", bufs=1) as consts,
            tc.tile_pool(name="x", bufs=bufs_x) as xpool,
            tc.tile_pool(name="cb", bufs=bufs_cb) as cbpool,
            tc.tile_pool(name="work", bufs=4) as sbuf,
            tc.tile_pool(name="accum", bufs=1) as accum,
            tc.tile_pool(name="pA", bufs=1, space="PSUM") as psumA,
            tc.tile_pool(name="pUE", bufs=2, space="PSUM") as psumUE,
            tc.tile_pool(name="pCB", bufs=2, space="PSUM") as psumCB,
        ):
            # prologue-critical consts ride the ACT HWDGE ring so the x
            # stream owns the SP ring from t=0
            x0t_sb = consts.tile([P, DT, pairs], R32)
            nc.scalar.dma_start(
                x0t_sb, x0t_t.ap().rearrange("(dt p) j -> p dt j", p=P)
            )
            m1_sb = consts.tile([P, DT, D], R32)
            nc.scalar.dma_start(
                m1_sb, m1_t.ap().rearrange("(dt p) d -> p dt d", p=P)
            )
            basis_sb = consts.tile([pairs, pairs, P], F16)
            nc.scalar.dma_start(
                basis_sb, basis_t.ap().rearrange("k (j p) -> k j p", p=P)
            )
            # tail-only consts: declared here, loaded late (after the loop)
            wvt_sb = consts.tile([P, DT, D], R32)
            sel_sb = consts.tile([selp, pairs], F32)
            ones2_sb = consts.tile([P, 2], F16)
            ident_sb = consts.tile([P, P], F32)

            # ---- phase A: c = x0 @ M1 for all pairs ----
            c_ps = psumA.tile([pairs, D], F32, tag="cA")
            for dt in range(DT):
                nc.tensor.matmul(
                    c_ps[:],
                    x0t_sb[:, dt, :],
                    m1_sb[:, dt, :],
                    start=(dt == 0),
                    stop=(dt == DT - 1),
                )
            c_all_sb = consts.tile([pairs, D], F16)
            nc.scalar.copy(c_all_sb, c_ps[:])

            # e_all[p, lt, j] = exp(s[j, l]) with l = p*LT + lt
            e_all = accum.tile([P, LT, pairs], F16)
            ue_flat = accum.tile([1, pairs, D], F32, name="ue_flat")

            if mode == "noscore":
                nc.vector.memset(e_all.rearrange("p a b -> p (a b)"), 0.01)

            _loop = contextlib.ExitStack()
            if hw_reps is not None:
                _loop.enter_context(tc.For_i(
                    0, hw_reps, 1,
                    hint_engines=(mybir.EngineType.PE,
                                  mybir.EngineType.Activation,
                                  mybir.EngineType.DVE),
                ))
            LEAD = lead   # cb broadcast runs this many pairs ahead
            x_tiles = {}
            cb_tiles = {}
            s_tiles = {}
            ue_tiles = {}

            def emit_bcast(j):
                cb_ps = psumCB.tile([P, D], F32, tag="cbp", name="cb_ps")
                nc.tensor.matmul(
                    cb_ps[:], basis_sb[:, j, :], c_all_sb[:],
                    start=True, stop=True,
                )
                cb = cbpool.tile([P, D], F16, tag="cb", name="cb",
                                 bufs=bufs_cb)
                nc.scalar.copy(cb, cb_ps[:])
                cb_tiles[j] = cb

            def emit_score(j, lt, x_sb, cb, s_out):
                eng = assign[lt]
                if eng == "D":
                    # fused mul+reduce on DVE (scalar_tensor_tensor)
                    scr = sbuf.tile([P, D], F16, tag="scrD", name="scrD",
                                    bufs=3)
                    nc.vector.scalar_tensor_tensor(
                        out=scr[:], in0=x_sb, scalar=1.0, in1=cb[:],
                        op0=MULT, op1=MULT, accum_out=s_out,
                    )
                elif eng in ("P", "A"):
                    # mult on Pool/DVE, free-axis reduce on ACT
                    prod = sbuf.tile([P, D], F16, tag=f"prod{eng}",
                                     name="prod", bufs=3)
                    if eng == "P":
                        nc.gpsimd.tensor_tensor(prod, x_sb, cb[:], MULT)
                    else:
                        nc.vector.tensor_tensor(prod, x_sb, cb[:], MULT)
                    act_scr = sbuf.tile([P, D], F16, tag="ascr",
                                        name="ascr", bufs=2)
                    nc.scalar.activation(
                        act_scr[:], prod,
                        mybir.ActivationFunctionType.Copy,
                        accum_out=s_out,
                    )
                else:
                    raise ValueError(f"bad assign {assign!r}")

            if mode not in ("dmaonly",):
                for j in range(min(LEAD, pairs)):
                    emit_bcast(j)

            for j in range(pairs):
                if j % dma_chunk == 0:
                    xc = xpool.tile([P, dma_chunk, LT, D], F16, tag="x",
                                    name="xc")
                    nc.sync.dma_start(
                        xc,
                        x_ap[j : j + dma_chunk].rearrange(
                            "c (p lt) d -> p c lt d", lt=LT
                        ),
                    )
                    for jc in range(dma_chunk):
                        x_tiles[j + jc] = xc[:, jc]
                x_sb = x_tiles[j]
                if mode == "dmaonly":
                    continue

                jc = j % 2  # exp/ue-copy batch over pair groups of 2
                j0 = j - jc
                if mode != "noscore":
                    if j + LEAD < pairs:
                        emit_bcast(j + LEAD)
                    cb = cb_tiles.pop(j)
                    if jc == 0:
                        s2 = sbuf.tile([P, 2, LT], F32, tag="s",
                                       bufs=bufs_s)
                        s_tiles[j0] = s2
                    s2 = s_tiles[j0]
                    for lt in range(LT):
                        emit_score(j, lt, x_sb[:, lt, :], cb,
                                   s2[:, jc, lt : lt + 1])
                    if jc == 1:
                        nc.scalar.activation(
                            e_all[:, 0:LT, j0 : j0 + 2],
                            s_tiles.pop(j0).rearrange("p c lt -> p lt c"),
                            mybir.ActivationFunctionType.Exp,
                        )

                if mode == "noue" or jc == 0:
                    continue
                # both pairs' e columns are now written; emit their ue
                ue_ps = psumUE.tile([1, 2, D], F32, tag="ue",
                                    name="ue_ps", bufs=2)
                for ji in range(2):
                    jj = j0 + ji
                    xj = x_tiles[jj]
                    for lt in range(LT):
                        nc.tensor.matmul(
                            ue_ps[0:1, ji, :],
                            e_all[:, lt, jj : jj + 1],
                            xj[:, lt, :],
                            start=(lt == 0),
                            stop=(lt == LT - 1),
                        )
                dst = ue_flat[0:1, j0 : j0 + 2, :]
                src = ue_ps[0:1, :, :]
                if ue_copy == "act":
                    nc.scalar.copy(dst, src)
                elif ue_copy == "pool":
                    nc.gpsimd.tensor_copy(dst, src)
                else:
                    nc.vector.tensor_copy(dst, src)
            _loop.close()

            if mode == "full":
                nc.scalar.dma_start(
                    wvt_sb, wvt_t.ap().rearrange("(dt p) d -> p dt d", p=P)
                )
                nc.scalar.dma_start(sel_sb, sel_t.ap())
                nc.scalar.dma_start(ones2_sb, ones2_t.ap())
                nc.scalar.dma_start(ident_sb, ident_t.ap())
                # ---- tail ----
                # z[j] = sum over (p, lt) of e_all
                zpart_ps = psumA.tile([selp, 2], F32, tag="tail")
                nc.tensor.matmul(
                    zpart_ps[:],
                    e_all[:, 0:LT, :].rearrange("p lt j -> p (lt j)"),
                    ones2_sb[:],
                )
                zpart_sb = sbuf.tile([selp, 1], F32, tag="zpart")
                nc.scalar.copy(zpart_sb, zpart_ps[:, 0:1])
                zcol_ps = psumA.tile([pairs, 1], F32, tag="tail")
                nc.tensor.matmul(zcol_ps[:], sel_sb[:], zpart_sb[:])
                zcol_sb = sbuf.tile([pairs, 1], F32, tag="zcol")
                nc.scalar.copy(zcol_sb, zcol_ps[:])
                zi_sb = sbuf.tile([pairs, 1], F32, tag="zi")
                nc.vector.reciprocal(zi_sb, zcol_sb)

                # flat ue rows -> [pairs, D] -> PE transpose -> project
                ue_rows = sbuf.tile([pairs, D], F32, tag="uerows")
                nc.sync.dma_start(
                    ue_rows,
                    ue_flat[0:1, :, :].rearrange("o j d -> o (j d)"),
                )
                uet_tail_ps = psumA.tile([P, DT, pairs], F32, tag="tail",
                                         name="uet_tail_ps")
                for dt in range(DT):
                    nc.tensor.transpose(
                        uet_tail_ps[:, dt, :],
                        ue_rows[:, ts(dt, P)],
                        ident_sb[:pairs, :pairs],
                    )
                uet_sb = sbuf.tile([P, DT, pairs], R32, tag="uetsb")
                nc.scalar.copy(uet_sb, uet_tail_ps[:, :, :])

                # out = (uet^T @ WvT) * zi
                out_ps = psumA.tile([pairs, D], F32, tag="cA")
                for dt in range(DT):
                    nc.tensor.matmul(
                        out_ps[:],
                        uet_sb[:, dt, :],
                        wvt_sb[:, dt, :],
                        start=(dt == 0),
                        stop=(dt == DT - 1),
                    )
                out_sb = sbuf.tile([pairs, D], F32, tag="out")
                nc.scalar.activation(
                    out_sb,
                    out_ps[:],
                    mybir.ActivationFunctionType.Copy,
                    scale=zi_sb[:],
                )
                nc.sync.dma_start(out_t.ap(), out_sb)
                if dbg:
                    dbg_c = sbuf.tile([pairs, D], F32, tag="dbgc")
                    nc.scalar.copy(dbg_c, c_all_sb[:])
                    nc.sync.dma_start(dbg_c_t.ap(), dbg_c)
                    dbg_e = sbuf.tile([P, LT * pairs], F32, tag="dbge")
                    nc.scalar.copy(
                        dbg_e, e_all.rearrange("p lt j -> p (lt j)")
                    )
                    nc.sync.dma_start(dbg_e_t.ap(), dbg_e)
                    nc.sync.dma_start(
                        dbg_ue_t.ap(),
                        ue_flat[0:1, :, :].rearrange("o j d -> o (j d)"),
                    )

    nc.compile()
    return nc


def _host_consts(pairs=PAIRS):
    ident = np.eye(P, dtype=np.float32)
    sel = np.zeros((LT * pairs, pairs), dtype=np.float32)
    for m in range(LT * pairs):
        sel[m, m % pairs] = 1.0
    ones2 = np.ones((P, 2), dtype=np.float16)
    basis = np.zeros((pairs, pairs, P), dtype=np.float16)
    for j in range(pairs):
        basis[j, j, :] = 1.0
    return ident, sel, ones2, basis.reshape(pairs, pairs * P)


def make_in_maps(inputs):
    x = np.asarray(inputs["x"], dtype=np.float32)
    Wq = np.asarray(inputs["Wq"], dtype=np.float32)
    Wk = np.asarray(inputs["Wk"], dtype=np.float32)
    Wv = np.asarray(inputs["Wv"], dtype=np.float32)

    temp = np.sqrt(np.float32(D)).astype(np.float64)
    m1 = ((Wq.T.astype(np.float64) @ Wk.astype(np.float64)) / temp).astype(
        np.float32
    )
    wvt = np.ascontiguousarray(Wv.T)
    ident, sel, ones2, basis = _host_consts()

    shards_f32 = x.reshape(N_CORES, PAIRS, L, D)
    shards = np.ascontiguousarray(shards_f32.astype(np.float16))
    return [
        {
            "x": shards[c],
            "x0t": np.ascontiguousarray(shards_f32[c][:, 0, :].T),
            "m1": m1,
            "wvt": wvt,
            "ident": ident,
            "sel": sel,
            "ones2": ones2,
            "basis": basis,
        }
        for c in range(N_CORES)
    ]


_NC_CACHE = {}


def kernel(x, Wq, Wk, Wv):
    if "nc" not in _NC_CACHE:
        _NC_CACHE["nc"] = _build_program()
    nc = _NC_CACHE["nc"]

    in_maps = make_in_maps({"x": x, "Wq": Wq, "Wk": Wk, "Wv": Wv})
    res = bass_utils.run_bass_kernel_spmd(
        nc, in_maps, core_ids=list(range(N_CORES)), trace=False
    )
    out = np.stack([res.results[c]["out"] for c in range(N_CORES)])
    return out.reshape(B, INST, D)
